# revision 24
# baseline (speedup 1.0000x reference)
"""Trainium2 Bass kernel for nn_CompressiveMemory (B=2,T=8192,D=1024,H=16,DK=DV=64,SEG=512).

Sharding: 8 cores; core c -> batch c//4, 4 consecutive heads 4*(c%4)..+4.
Due to the PyTorch .view head split, head h of segment g corresponds to token
rows [g*512 + h*32, g*512 + (h+1)*32), so each core owns one contiguous
128-row block per segment. Host pre-transposes x and weights; device keeps
activations feature-on-partition and uses a permuted in-segment token order
s' = half*256 + c*32 + i (s = i*16 + (2c+half)), which softmax/attention is
equivariant to.
"""

import numpy as np

H, DK, DV, SEG, D = 16, 64, 64, 512, 1024
B, T = 2, 8192
NSEG = T // SEG          # 16
NCORES = 8
HPC = 4                  # heads per core
NSB = NSEG // 2          # super-blocks of 2 segments

_cache = {}


def _build_program(probe=""):
    import concourse.bass as bass
    import concourse.tile as tile
    from concourse import bacc, mybir

    fp16 = mybir.dt.float16
    f32 = mybir.dt.float32
    AF = mybir.ActivationFunctionType
    OP = mybir.AluOpType

    nc = bacc.Bacc("TRN2", target_bir_lowering=False)

    xsT_d = nc.dram_tensor("xst", [NSB, D, 256], fp16, kind="ExternalInput")
    wqkv_d = nc.dram_tensor("wqkv", [3, D, D], fp16, kind="ExternalInput")
    wout_d = nc.dram_tensor("wout", [D, D], fp16, kind="ExternalInput")
    gates_d = nc.dram_tensor("gates", [128, 2, 2], f32, kind="ExternalInput")
    y_d = nc.dram_tensor("y", [NSEG, 128, D], f32, kind="ExternalOutput")

    with tile.TileContext(nc) as tc:
        with (
            tc.tile_pool(name="static", bufs=1) as stp_,
            tc.tile_pool(name="ps", bufs=1, space="PSUM") as psp,
            tc.tile_pool(name="xp", bufs=2) as xp,
            tc.tile_pool(name="qkv", bufs=2) as qkvp,
            tc.tile_pool(name="work", bufs=2) as wp,
            tc.tile_pool(name="pt", bufs=2) as ptp,
            tc.tile_pool(name="outp", bufs=2) as op_,
        ):
            # ---- static tiles ----
            wq = stp_.tile([128, 8, D], fp16, tag="wq", name="wq")
            wk = stp_.tile([128, 8, D], fp16, tag="wk", name="wk")
            wv = stp_.tile([128, 8, D], fp16, tag="wv", name="wv")
            wo = stp_.tile([128, 8, D], fp16, tag="wo", name="wo")
            for dc_ in range(8):
                nc.sync.dma_start(
                    wq[:, dc_, :],
                    wqkv_d[0, dc_ * 128:(dc_ + 1) * 128, :])
            for t, i in ((wk, 1), (wv, 2)):
                nc.sync.dma_start(t, wqkv_d[i].rearrange("(c p) o -> p c o", p=128))
            nc.sync.dma_start(wo, wout_d[:].rearrange("(c p) o -> p c o", p=128))

            gates = stp_.tile([128, 2, 2], f32, tag="gates", name="gates")
            nc.sync.dma_start(gates, gates_d[:])

            ones_h = stp_.tile([128, 64], fp16, tag="ones_h", name="ones_h")
            nc.vector.memset(ones_h, 1.0)

            memF = stp_.tile([128, 2, 64], f32, tag="memF", name="memF")
            nc.vector.memset(memF, 0.0)
            memt = stp_.tile([128, 2, 64], fp16, tag="memt", name="memt")
            nc.vector.memset(memt, 0.0)
            Zt = stp_.tile([128, 2], f32, tag="Zt", name="Zt")
            nc.vector.memset(Zt, 0.0)

            qkvT, qqs, vns, kns, sks, sqs = {}, {}, {}, {}, {}, {}

            def shuffle_q(sb2):
                qT2 = qkvT[sb2][0]
                qq = wp.tile([128, 2, 2, 2, 512], fp16, tag="qq", name=f"qq{sb2}")
                qqs[sb2] = qq
                for hs in range(2):
                    hp = slice(hs * 64, (hs + 1) * 64)
                    cs = slice(hs * 256, (hs + 1) * 256)
                    nc.sync.dma_start(qq[0:64, :, :, :, cs], qT2[hp, :, :, :, :])
                    nc.sync.dma_start(qq[64:128, :, :, 0, cs],
                                      qT2[hp, :, 1:4:2, :, :])
                    nc.sync.dma_start(qq[64:128, :, :, 1, cs],
                                      qT2[hp, :, 0:4:2, :, :])
            def shuffle_q_elem(sb2):
                qq = qqs[sb2]
                sqC = wp.tile([128, 2, 2, 512], fp16, tag="sqC", name=f"sqC{sb2}")
                sqs[sb2] = sqC
                for s2 in range(2):
                    for u2 in range(2):
                        q1 = qq[:, s2, u2, 0, :]
                        sqE = wp.tile([128, 512], fp16, tag="sqE",
                                      name=f"sqE{sb2}{s2}{u2}")
                        sqR = wp.tile([128, 512], fp16, tag="sqR",
                                      name=f"sqR{sb2}{s2}{u2}")
                        nc.scalar.activation(sqE, q1, AF.Exp)
                        nc.vector.tensor_scalar(sqE, sqE, 1.0, None, OP.min)
                        nc.scalar.activation(sqR, q1, AF.Relu)
                        nc.vector.tensor_tensor(sqC[:, s2, u2, :], sqE, sqR, OP.add)

            def shuffle_k(sb2):
                kT2 = qkvT[sb2][1]
                kn = wp.tile([128, 2, 2, 4, 2, 64], fp16, tag="kn", name=f"kn{sb2}")
                kns[sb2] = kn
                for hs in range(2):
                    hp = slice(hs * 64, (hs + 1) * 64)
                    nc.sync.dma_start_transpose(
                        kn[:, hs, :, :, :, :], kT2[hp, :, :, :, :])

            def shuffle_k_elem(sb2):
                kn = kns[sb2]
                skn = wp.tile([128, 2, 2, 4, 2, 64], fp16, tag="skn",
                              name=f"skn{sb2}")
                sks[sb2] = skn
                knf = kn.rearrange("p a b c d e -> p (a b c d e)")
                ke = wp.tile([128, 2048], fp16, tag="ke", bufs=1, name=f"ke{sb2}")
                kr = wp.tile([128, 2048], fp16, tag="kr", bufs=1, name=f"kr{sb2}")
                nc.scalar.activation(ke, knf, AF.Exp)
                nc.vector.tensor_scalar(ke, ke, 1.0, None, OP.min)
                nc.scalar.activation(kr, knf, AF.Relu)
                nc.vector.tensor_tensor(
                    skn.rearrange("p a b c d e -> p (a b c d e)"), ke, kr, OP.add)

            def shuffle_v(sb2):
                vT2 = qkvT[sb2][2]
                vn = wp.tile([128, 2, 2, 4, 2, 80], fp16, tag="vn", name=f"vn{sb2}")
                vns[sb2] = vn
                nc.vector.memset(
                    vn[:, :, :, :, :, 64:65].rearrange(
                        "p a b c d e -> p (a b c d e)"), 1.0)
                for hs in range(2):
                    hp = slice(hs * 64, (hs + 1) * 64)
                    nc.sync.dma_start_transpose(
                        vn[:, hs, :, :, :, 0:64], vT2[hp, :, :, :, :])

            # projection chunk machinery: q-major so qq shuffles can fire early
            pstate = {"chunks": [], "done": 0, "sb2": None}

            def start_proj(sb2):
                xt = xp.tile([128, 8, 256], fp16, tag="xt", name=f"xt{sb2}")
                nc.gpsimd.dma_start(xt, xsT_d[sb2].rearrange("(c p) r -> p c r",
                                                             p=128))
                qT2 = qkvp.tile([128, 2, HPC, 8, 32], fp16, tag="qT", bufs=1,
                                name=f"qT{sb2}")
                kT2 = qkvp.tile([128, 2, HPC, 8, 32], fp16, tag="kT",
                                name=f"kT{sb2}")
                vT2 = qkvp.tile([128, 2, HPC, 8, 32], fp16, tag="vT", bufs=1,
                                name=f"vT{sb2}")
                qkvT[sb2] = (qT2, kT2, vT2)
                chunks = []
                for wi, (wt, dst) in enumerate(((wq, qT2), (wk, kT2), (wv, vT2))):
                    for m in range(8):
                        def chunk(wi=wi, wt=wt, dst=dst, m=m):
                            ps = psp.tile([128, 512], f32, tag="st", bufs=3,
                                          name=f"pj{sb2}_{wi}_{m}")
                            for dc in range(8):
                                nc.tensor.matmul(
                                    ps[:, 0:256],
                                    lhsT=wt[:, dc, m * 128:(m + 1) * 128],
                                    rhs=xt[:, dc, :],
                                    start=(dc == 0), stop=(dc == 7),
                                )
                            pp = ps[:, 0:256].rearrange(
                                "p (s h i) -> p s h i", s=2, h=HPC)
                            if wi == 0:
                                nc.scalar.copy(dst[:, :, :, m, :], pp)
                            else:
                                nc.vector.tensor_copy(dst[:, :, :, m, :], pp)
                        chunks.append(chunk)
                pstate["chunks"] = chunks
                pstate["done"] = 0
                pstate["sb2"] = sb2

            def emit_chunks(n):
                while n > 0 and pstate["chunks"]:
                    pstate["chunks"].pop(0)()
                    pstate["done"] += 1
                    if pstate["done"] == 8:
                        shuffle_q(pstate["sb2"])
                    elif pstate["done"] == 16:
                        shuffle_k(pstate["sb2"])
                    elif pstate["done"] == 24:
                        shuffle_k_elem(pstate["sb2"])
                        shuffle_q_elem(pstate["sb2"])
                        shuffle_v(pstate["sb2"])
                    n -= 1

            pending_out = []

            def do_outproj():
                attBT_p, g_p = pending_out.pop(0)
                yt = op_.tile([128, D], f32, tag="yt", bufs=1, name=f"yt{g_p}")
                for do in range(2):
                    yp = psp.tile([128, 512], f32, tag="att", bufs=5,
                                  name=f"yp{g_p}{do}")
                    for c2 in range(8):
                        nc.tensor.matmul(yp, lhsT=attBT_p[:, c2, :],
                                         rhs=wo[:, c2, do * 512:(do + 1) * 512],
                                         start=(c2 == 0), stop=(c2 == 7))
                    nc.scalar.copy(yt[:, do * 512:(do + 1) * 512], yp)
                nc.gpsimd.dma_start(y_d[g_p], yt)

            # ---- phase 1 (ST matmuls + exp), hoistable one segment ahead ----
            p1cache = {}

            def do_phase1(sbp, sp_):
                kT = qkvT[sbp][1]
                qq2 = qqs[sbp]
                g1 = 2 * sbp + sp_
                pts = {}
                for u in range(2):
                    hA, hB = 2 * u, 2 * u + 1
                    ptA = ptp.tile([128, 4, 512], fp16, tag="ptA",
                                   name=f"ptA{g1}{u}")
                    ptB = ptp.tile([128, 4, 512], fp16, tag="ptB",
                                   name=f"ptB{g1}{u}")
                    pts[u] = (ptA, ptB)
                    for tc_i in range(4):
                        hs, cg = tc_i // 2, tc_i % 2
                        hp = slice(hs * 64, (hs + 1) * 64)
                        cgs = slice(cg * 4, (cg + 1) * 4)
                        stp = psp.tile([128, 512], f32, tag="st", bufs=3,
                                       name=f"stA{g1}{u}{tc_i}")
                        nc.tensor.matmul(
                            stp, lhsT=kT[hp, sp_, hA, cgs],
                            rhs=qq2[hp, sp_, u, hs, :],
                            start=True, stop=True,
                        )
                        nc.scalar.activation(ptA[:, tc_i], stp, AF.Exp,
                                             scale=0.125)
                        tc_j = tc_i ^ 2
                        hs2, cg2 = tc_j // 2, tc_j % 2
                        hp2 = slice(hs2 * 64, (hs2 + 1) * 64)
                        cgs2 = slice(cg2 * 4, (cg2 + 1) * 4)
                        stpB = psp.tile([128, 512], f32, tag="st", bufs=3,
                                        name=f"stB{g1}{u}{tc_i}")
                        nc.tensor.matmul(
                            stpB, lhsT=kT[hp2, sp_, hB, cgs2],
                            rhs=qq2[hp2, sp_, u, 1 - hs2, :],
                            start=True, stop=True,
                        )
                        nc.scalar.activation(ptB[:, tc_j], stpB, AF.Exp,
                                             scale=0.125)
                p1cache[g1] = pts

            start_proj(0)
            emit_chunks(24)
            for sb in range(NSB):
                vn2, kn2, skn2, sqC2 = vns[sb], kns[sb], sks[sb], sqs[sb]
                if sb + 1 < NSB:
                    start_proj(sb + 1)
                for s in range(2):
                    g_seg = 2 * sb + s
                    attBT = op_.tile([128, 8, 128], fp16, tag="attBT",
                                     name=f"aBT{g_seg}")
                    if len(pending_out) > 1:
                        do_outproj()

                    do_phase1(sb, s)
                    emit_chunks(8 if s == 0 else 0)

                    # ---- phase 2: mem-update matmuls + state updates ----
                    mups = {}
                    for u in range(2):
                        mup = psp.tile([128, 512], f32, tag="att", bufs=5,
                                       name=f"mup{g_seg}{u}")
                        mups[u] = mup
                        for tc_i in range(4):
                            st = (tc_i == 0)
                            sp = (tc_i == 3)
                            hs, cg = tc_i // 2, tc_i % 2
                            vA = vn2[:, hs, s, 2 * u, cg]
                            vB = vn2[:, hs, s, 2 * u + 1, cg]
                            nc.tensor.matmul(mup[0:64, 0:65],
                                             lhsT=skn2[:, hs, s, 2 * u, cg],
                                             rhs=vA[:, 0:65], start=st, stop=sp,
                                             tile_position=(0, 0))
                            nc.tensor.matmul(mup[64:128, 0:65],
                                             lhsT=skn2[:, hs, s, 2 * u + 1, cg],
                                             rhs=vB[:, 0:65], start=st, stop=sp,
                                             tile_position=(0, 64))
                        nc.vector.tensor_tensor(Zt[:, u:u + 1], Zt[:, u:u + 1],
                                                mup[:, 64:65], OP.add)
                    emit_chunks(8 if s == 0 else 0)

                    pts = p1cache.pop(g_seg)
                    # ---- phase 3: per-unit mem/rsum + dot/den + blend ----
                    for u in range(2):
                        hA, hB = 2 * u, 2 * u + 1
                        sqC = sqC2[:, s, u, :]
                        ptA, ptB = pts[u]
                        mup = mups[u]
                        memp = psp.tile([128, 512], f32, tag="att", bufs=5,
                                        name=f"mem{g_seg}{u}")
                        rsmp = psp.tile([128, 512], f32, tag="att", bufs=5,
                                        name=f"rsm{g_seg}{u}")
                        nc.tensor.matmul(memp[0:64, :], lhsT=memt[0:64, u, :],
                                         rhs=sqC[0:64, :], start=True, stop=True)
                        nc.tensor.matmul(memp[64:128, :], lhsT=memt[64:128, u, :],
                                         rhs=sqC[64:128, :], start=True, stop=True,
                                         tile_position=(64, 64))
                        nc.tensor.matmul(rsmp[0:64, :], lhsT=ones_h[0:64, :],
                                         rhs=sqC[0:64, :], start=True, stop=True)
                        nc.tensor.matmul(rsmp[64:128, :], lhsT=ones_h[64:128, :],
                                         rhs=sqC[64:128, :], start=True, stop=True,
                                         tile_position=(64, 64))
                        # mem/Z state update for next segment
                        nc.vector.tensor_tensor(memF[:, u, :], memF[:, u, :],
                                                mup[:, 0:64], OP.add)
                        nc.gpsimd.tensor_copy(memt[:, u, :], memF[:, u, :])
                        # gZ[p] = g[p] / Zt[p]; 1/rsum via fast approx
                        rzt = wp.tile([128, 1], f32, tag="rzt", bufs=2,
                                      name=f"rzt{g_seg}{u}")
                        nc.vector.reciprocal_approx_fast(rzt, Zt[:, u:u + 1])
                        gZ = wp.tile([128, 1], f32, tag="gZ", bufs=2,
                                     name=f"gZ{g_seg}{u}")
                        nc.vector.tensor_tensor(gZ, gates[:, u, 0:1], rzt,
                                                OP.mult)
                        recR = wp.tile([128, 512], f32, tag="rz", bufs=2,
                                       name=f"rR{g_seg}{u}")
                        nc.vector.reciprocal_approx_fast(recR, rsmp)
                        # dot + den matmuls
                        dotp = psp.tile([128, 512], f32, tag="att", bufs=5,
                                        name=f"dot{g_seg}{u}")
                        denp = psp.tile([128, 512], f32, tag="att", bufs=5,
                                        name=f"den{g_seg}{u}")
                        for tc_i in range(4):
                            st = (tc_i == 0)
                            sp = (tc_i == 3)
                            hs, cg = tc_i // 2, tc_i % 2
                            vA = vn2[:, hs, s, 2 * u, cg]
                            vB = vn2[:, hs, s, 2 * u + 1, cg]
                            nc.tensor.matmul(dotp[0:64, :], lhsT=vA[:, 0:64],
                                             rhs=ptA[:, tc_i], start=st, stop=sp)
                            nc.tensor.matmul(dotp[64:128, :], lhsT=vB[:, 0:64],
                                             rhs=ptB[:, tc_i], start=st, stop=sp,
                                             tile_position=(0, 64))
                            nc.tensor.matmul(denp[0:64, :], lhsT=ones_h,
                                             rhs=ptA[:, tc_i], start=st, stop=sp)
                            nc.tensor.matmul(denp[64:128, :], lhsT=ones_h,
                                             rhs=ptB[:, tc_i], start=st, stop=sp,
                                             tile_position=(0, 64))
                        # blend
                        recD = wp.tile([128, 512], f32, tag="recD", bufs=2,
                                       name=f"rD{g_seg}{u}")
                        nc.vector.reciprocal_approx_fast(recD, denp)
                        t1b = wp.tile([128, 512], f32, tag="t1b", bufs=2,
                                      name=f"t1b{g_seg}{u}")
                        nc.vector.scalar_tensor_tensor(
                            t1b, dotp, gates[:, u, 1:2], recD, OP.mult, OP.mult)
                        t2b = wp.tile([128, 512], f32, tag="t2b", bufs=2,
                                      name=f"t2b{g_seg}{u}")
                        nc.vector.scalar_tensor_tensor(
                            t2b, memp, gZ, recR, OP.mult, OP.mult)
                        attU = wp.tile([128, 512], fp16, tag="attU",
                                       name=f"aU{g_seg}{u}")
                        nc.vector.tensor_tensor(attU, t1b, t2b, OP.add)
                        # assembly into attBT
                        rcA = slice(hA * 32, hA * 32 + 32)
                        rcB = slice(hB * 32, hB * 32 + 32)
                        nc.gpsimd.tensor_copy(
                            attBT[0:64, :, rcA],
                            attU[0:64, 0:256].rearrange("p (c i) -> p c i", c=8))
                        nc.gpsimd.tensor_copy(
                            attBT[64:128, :, rcB],
                            attU[64:128, 256:512].rearrange("p (c i) -> p c i",
                                                            c=8))
                        nc.sync.dma_start(
                            attBT[64:128, :, rcA],
                            attU[0:64, 256:512].rearrange("p (c i) -> p c i",
                                                          c=8))
                        nc.sync.dma_start(
                            attBT[0:64, :, rcB],
                            attU[64:128, 0:256].rearrange("p (c i) -> p c i",
                                                          c=8))
                        emit_chunks(4 if s == 0 else 0)
                    pending_out.append((attBT, g_seg))

            while pending_out:
                do_outproj()

    nc.compile()
    return nc


def _prep_inputs(x, Wq, Wk, Wv, Wout, betas):
    gate = 1.0 / (1.0 + np.exp(-np.asarray(betas, np.float32)))[0, :, 0, :]  # (H, DV)
    wqkv = np.ascontiguousarray(
        np.stack([Wq.T, Wk.T, Wv.T]).astype(np.float16))
    wout = np.ascontiguousarray(Wout.T.astype(np.float16))
    in_maps = []
    for c in range(NCORES):
        b, h0 = c // 4, HPC * (c % 4)
        xs = x[b].reshape(NSEG, SEG, D)[:, h0 * 32:h0 * 32 + 128, :]  # (16,128,D)
        xsT = np.ascontiguousarray(xs.transpose(0, 2, 1))             # (16,D,128)
        xst = np.ascontiguousarray(
            xsT.reshape(NSB, 2, D, 128).transpose(0, 2, 1, 3).reshape(NSB, D, 256)
        ).astype(np.float16)
        gts = np.zeros((128, 2, 2), np.float32)
        for u in range(2):
            for p in range(128):
                hd = h0 + 2 * u + p // 64
                gts[p, u, 0] = gate[hd, p % 64]
                gts[p, u, 1] = 1.0 - gate[hd, p % 64]
        in_maps.append({"xst": xst, "wqkv": wqkv, "wout": wout, "gates": gts})
    return in_maps


def kernel(x, Wq, Wk, Wv, Wout, betas):
    from concourse.bass_utils import run_bass_kernel_spmd

    x = np.asarray(x, np.float32)
    in_maps = _prep_inputs(x, np.asarray(Wq), np.asarray(Wk), np.asarray(Wv),
                           np.asarray(Wout), betas)
    if "nc" not in _cache:
        _cache["nc"] = _build_program()
    res = run_bass_kernel_spmd(_cache["nc"], in_maps, core_ids=list(range(NCORES)))
    out = np.zeros((B, T, D), np.float32)
    for c in range(NCORES):
        b, h0 = c // 4, HPC * (c % 4)
        yc = res.results[c]["y"]  # (16, 128, D)
        for g in range(NSEG):
            r0 = g * SEG + h0 * 32
            out[b, r0:r0 + 128, :] = yc[g]
    return out



# revision 25
# speedup vs baseline: 1.0615x; 1.0615x over previous
"""Trainium2 Bass kernel for nn_CompressiveMemory (B=2,T=8192,D=1024,H=16,DK=DV=64,SEG=512).

Sharding: 8 cores; core c -> batch c//4, 4 consecutive heads 4*(c%4)..+4.
Due to the PyTorch .view head split, head h of segment g corresponds to token
rows [g*512 + h*32, g*512 + (h+1)*32), so each core owns one contiguous
128-row block per segment. Host pre-transposes x and weights; device keeps
activations feature-on-partition and uses a permuted in-segment token order
s' = half*256 + c*32 + i (s = i*16 + (2c+half)), which softmax/attention is
equivariant to.
"""

import numpy as np

H, DK, DV, SEG, D = 16, 64, 64, 512, 1024
B, T = 2, 8192
NSEG = T // SEG          # 16
NCORES = 8
HPC = 4                  # heads per core
NSB = NSEG // 2          # super-blocks of 2 segments

_cache = {}


def _build_program(probe=""):
    import concourse.bass as bass
    import concourse.tile as tile
    from concourse import bacc, mybir

    fp16 = mybir.dt.float16
    f32 = mybir.dt.float32
    AF = mybir.ActivationFunctionType
    OP = mybir.AluOpType

    nc = bacc.Bacc("TRN2", target_bir_lowering=False)

    xsT_d = nc.dram_tensor("xst", [NSB, D, 256], fp16, kind="ExternalInput")
    wqkv_d = nc.dram_tensor("wqkv", [3, D, D], fp16, kind="ExternalInput")
    wout_d = nc.dram_tensor("wout", [D, D], fp16, kind="ExternalInput")
    gates_d = nc.dram_tensor("gates", [128, 2, 2], f32, kind="ExternalInput")
    y_d = nc.dram_tensor("y", [NSEG, 128, D], f32, kind="ExternalOutput")

    with tile.TileContext(nc) as tc:
        with (
            tc.tile_pool(name="static", bufs=1) as stp_,
            tc.tile_pool(name="ps", bufs=1, space="PSUM") as psp,
            tc.tile_pool(name="xp", bufs=2) as xp,
            tc.tile_pool(name="qkv", bufs=2) as qkvp,
            tc.tile_pool(name="work", bufs=2) as wp,
            tc.tile_pool(name="pt", bufs=2) as ptp,
            tc.tile_pool(name="outp", bufs=2) as op_,
        ):
            # ---- static tiles ----
            wq = stp_.tile([128, 8, D], fp16, tag="wq", name="wq")
            wk = stp_.tile([128, 8, D], fp16, tag="wk", name="wk")
            wv = stp_.tile([128, 8, D], fp16, tag="wv", name="wv")
            wo = stp_.tile([128, 8, D], fp16, tag="wo", name="wo")
            for dc_ in range(8):
                nc.sync.dma_start(
                    wq[:, dc_, :],
                    wqkv_d[0, dc_ * 128:(dc_ + 1) * 128, :])
            for t, i in ((wk, 1), (wv, 2)):
                nc.sync.dma_start(t, wqkv_d[i].rearrange("(c p) o -> p c o", p=128))
            nc.sync.dma_start(wo, wout_d[:].rearrange("(c p) o -> p c o", p=128))

            gates = stp_.tile([128, 2, 2], f32, tag="gates", name="gates")
            nc.sync.dma_start(gates, gates_d[:])

            ones_h = stp_.tile([128, 64], fp16, tag="ones_h", name="ones_h")
            nc.vector.memset(ones_h, 1.0)

            memF = stp_.tile([128, 2, 64], f32, tag="memF", name="memF")
            nc.vector.memset(memF, 0.0)
            memt = stp_.tile([128, 2, 64], fp16, tag="memt", name="memt")
            nc.vector.memset(memt, 0.0)
            Zt = stp_.tile([128, 2], f32, tag="Zt", name="Zt")
            nc.vector.memset(Zt, 0.0)

            qkvT, qqs, vns, kns, sks, sqs = {}, {}, {}, {}, {}, {}

            def shuffle_q(sb2):
                qT2 = qkvT[sb2][0]
                qq = wp.tile([128, 2, 2, 2, 512], fp16, tag="qq", name=f"qq{sb2}")
                qqs[sb2] = qq
                for hs in range(2):
                    hp = slice(hs * 64, (hs + 1) * 64)
                    cs = slice(hs * 256, (hs + 1) * 256)
                    nc.sync.dma_start(qq[0:64, :, :, :, cs], qT2[hp, :, :, :, :])
                    nc.sync.dma_start(qq[64:128, :, :, 0, cs],
                                      qT2[hp, :, 1:4:2, :, :])
                    nc.sync.dma_start(qq[64:128, :, :, 1, cs],
                                      qT2[hp, :, 0:4:2, :, :])
                sqC = wp.tile([128, 2, 2, 512], fp16, tag="sqC", name=f"sqC{sb2}")
                sqs[sb2] = sqC
                for s2 in range(2):
                    for u2 in range(2):
                        q1 = qq[:, s2, u2, 0, :]
                        sqE = wp.tile([128, 512], fp16, tag="sqE",
                                      name=f"sqE{sb2}{s2}{u2}")
                        sqR = wp.tile([128, 512], fp16, tag="sqR",
                                      name=f"sqR{sb2}{s2}{u2}")
                        nc.scalar.activation(sqE, q1, AF.Exp)
                        nc.vector.tensor_scalar(sqE, sqE, 1.0, None, OP.min)
                        nc.scalar.activation(sqR, q1, AF.Relu)
                        nc.vector.tensor_tensor(sqC[:, s2, u2, :], sqE, sqR, OP.add)

            def shuffle_k(sb2):
                kT2 = qkvT[sb2][1]
                kn = wp.tile([128, 2, 2, 4, 2, 64], fp16, tag="kn", name=f"kn{sb2}")
                kns[sb2] = kn
                for hs in range(2):
                    hp = slice(hs * 64, (hs + 1) * 64)
                    nc.sync.dma_start_transpose(
                        kn[:, hs, :, :, :, :], kT2[hp, :, :, :, :])

            def shuffle_k_elem(sb2):
                kn = kns[sb2]
                skn = wp.tile([128, 2, 2, 4, 2, 64], fp16, tag="skn",
                              name=f"skn{sb2}")
                sks[sb2] = skn
                knf = kn.rearrange("p a b c d e -> p (a b c d e)")
                ke = wp.tile([128, 2048], fp16, tag="ke", bufs=1, name=f"ke{sb2}")
                kr = wp.tile([128, 2048], fp16, tag="kr", bufs=1, name=f"kr{sb2}")
                nc.scalar.activation(ke, knf, AF.Exp)
                nc.vector.tensor_scalar(ke, ke, 1.0, None, OP.min)
                nc.scalar.activation(kr, knf, AF.Relu)
                nc.vector.tensor_tensor(
                    skn.rearrange("p a b c d e -> p (a b c d e)"), ke, kr, OP.add)

            def shuffle_v(sb2):
                vT2 = qkvT[sb2][2]
                vn = wp.tile([128, 2, 2, 4, 2, 80], fp16, tag="vn", name=f"vn{sb2}")
                vns[sb2] = vn
                nc.vector.memset(
                    vn[:, :, :, :, :, 64:65].rearrange(
                        "p a b c d e -> p (a b c d e)"), 1.0)
                for hs in range(2):
                    hp = slice(hs * 64, (hs + 1) * 64)
                    nc.sync.dma_start_transpose(
                        vn[:, hs, :, :, :, 0:64], vT2[hp, :, :, :, :])

            # projection chunk machinery: q-major so qq shuffles can fire early
            pstate = {"chunks": [], "done": 0, "sb2": None}

            def start_proj(sb2):
                xt = xp.tile([128, 8, 256], fp16, tag="xt", name=f"xt{sb2}")
                nc.gpsimd.dma_start(xt, xsT_d[sb2].rearrange("(c p) r -> p c r",
                                                             p=128))
                qT2 = qkvp.tile([128, 2, HPC, 8, 32], fp16, tag="qT", bufs=1,
                                name=f"qT{sb2}")
                kT2 = qkvp.tile([128, 2, HPC, 8, 32], fp16, tag="kT",
                                name=f"kT{sb2}")
                vT2 = qkvp.tile([128, 2, HPC, 8, 32], fp16, tag="vT", bufs=1,
                                name=f"vT{sb2}")
                qkvT[sb2] = (qT2, kT2, vT2)
                chunks = []
                for wi, (wt, dst) in enumerate(((wq, qT2), (wk, kT2), (wv, vT2))):
                    for m in range(8):
                        def chunk(wi=wi, wt=wt, dst=dst, m=m):
                            ps = psp.tile([128, 512], f32, tag="st", bufs=3,
                                          name=f"pj{sb2}_{wi}_{m}")
                            for dc in range(8):
                                nc.tensor.matmul(
                                    ps[:, 0:256],
                                    lhsT=wt[:, dc, m * 128:(m + 1) * 128],
                                    rhs=xt[:, dc, :],
                                    start=(dc == 0), stop=(dc == 7),
                                )
                            pp = ps[:, 0:256].rearrange(
                                "p (s h i) -> p s h i", s=2, h=HPC)
                            if wi == 0:
                                nc.scalar.copy(dst[:, :, :, m, :], pp)
                            else:
                                nc.vector.tensor_copy(dst[:, :, :, m, :], pp)
                        chunks.append(chunk)
                pstate["chunks"] = chunks
                pstate["done"] = 0
                pstate["sb2"] = sb2

            def emit_chunks(n):
                while n > 0 and pstate["chunks"]:
                    pstate["chunks"].pop(0)()
                    pstate["done"] += 1
                    if pstate["done"] == 8:
                        shuffle_q(pstate["sb2"])
                    elif pstate["done"] == 16:
                        shuffle_k(pstate["sb2"])
                    elif pstate["done"] == 24:
                        shuffle_k_elem(pstate["sb2"])
                        shuffle_v(pstate["sb2"])
                    n -= 1

            pending_out = []

            def do_outproj():
                attBT_p, g_p = pending_out.pop(0)
                yt = op_.tile([128, D], f32, tag="yt", bufs=1, name=f"yt{g_p}")
                for do in range(2):
                    yp = psp.tile([128, 512], f32, tag="att", bufs=5,
                                  name=f"yp{g_p}{do}")
                    for c2 in range(8):
                        nc.tensor.matmul(yp, lhsT=attBT_p[:, c2, :],
                                         rhs=wo[:, c2, do * 512:(do + 1) * 512],
                                         start=(c2 == 0), stop=(c2 == 7))
                    nc.scalar.copy(yt[:, do * 512:(do + 1) * 512], yp)
                nc.gpsimd.dma_start(y_d[g_p], yt)

            # ---- phase 1 (ST matmuls + exp), hoistable one segment ahead ----
            p1cache = {}

            def do_phase1(sbp, sp_):
                kT = qkvT[sbp][1]
                qq2 = qqs[sbp]
                g1 = 2 * sbp + sp_
                pts = {}
                for u in range(2):
                    hA, hB = 2 * u, 2 * u + 1
                    ptA = ptp.tile([128, 4, 512], fp16, tag="ptA",
                                   name=f"ptA{g1}{u}")
                    ptB = ptp.tile([128, 4, 512], fp16, tag="ptB",
                                   name=f"ptB{g1}{u}")
                    pts[u] = (ptA, ptB)
                    for tc_i in range(4):
                        hs, cg = tc_i // 2, tc_i % 2
                        hp = slice(hs * 64, (hs + 1) * 64)
                        cgs = slice(cg * 4, (cg + 1) * 4)
                        stp = psp.tile([128, 512], f32, tag="st", bufs=3,
                                       name=f"stA{g1}{u}{tc_i}")
                        nc.tensor.matmul(
                            stp, lhsT=kT[hp, sp_, hA, cgs],
                            rhs=qq2[hp, sp_, u, hs, :],
                            start=True, stop=True,
                        )
                        nc.scalar.activation(ptA[:, tc_i], stp, AF.Exp,
                                             scale=0.125)
                        tc_j = tc_i ^ 2
                        hs2, cg2 = tc_j // 2, tc_j % 2
                        hp2 = slice(hs2 * 64, (hs2 + 1) * 64)
                        cgs2 = slice(cg2 * 4, (cg2 + 1) * 4)
                        stpB = psp.tile([128, 512], f32, tag="st", bufs=3,
                                        name=f"stB{g1}{u}{tc_i}")
                        nc.tensor.matmul(
                            stpB, lhsT=kT[hp2, sp_, hB, cgs2],
                            rhs=qq2[hp2, sp_, u, 1 - hs2, :],
                            start=True, stop=True,
                        )
                        nc.scalar.activation(ptB[:, tc_j], stpB, AF.Exp,
                                             scale=0.125)
                p1cache[g1] = pts

            start_proj(0)
            emit_chunks(24)
            for sb in range(NSB):
                vn2, kn2, skn2, sqC2 = vns[sb], kns[sb], sks[sb], sqs[sb]
                if sb + 1 < NSB:
                    start_proj(sb + 1)
                for s in range(2):
                    g_seg = 2 * sb + s
                    attBT = op_.tile([128, 8, 128], fp16, tag="attBT",
                                     name=f"aBT{g_seg}")
                    if len(pending_out) > 1:
                        do_outproj()

                    do_phase1(sb, s)
                    emit_chunks(8 if s == 0 else 0)

                    # ---- phase 2: mem-update matmuls + state updates ----
                    mups = {}
                    for u in range(2):
                        mup = psp.tile([128, 512], f32, tag="att", bufs=5,
                                       name=f"mup{g_seg}{u}")
                        mups[u] = mup
                        for tc_i in range(4):
                            st = (tc_i == 0)
                            sp = (tc_i == 3)
                            hs, cg = tc_i // 2, tc_i % 2
                            vA = vn2[:, hs, s, 2 * u, cg]
                            vB = vn2[:, hs, s, 2 * u + 1, cg]
                            nc.tensor.matmul(mup[0:64, 0:65],
                                             lhsT=skn2[:, hs, s, 2 * u, cg],
                                             rhs=vA[:, 0:65], start=st, stop=sp,
                                             tile_position=(0, 0))
                            nc.tensor.matmul(mup[64:128, 0:65],
                                             lhsT=skn2[:, hs, s, 2 * u + 1, cg],
                                             rhs=vB[:, 0:65], start=st, stop=sp,
                                             tile_position=(0, 64))
                        nc.vector.tensor_tensor(Zt[:, u:u + 1], Zt[:, u:u + 1],
                                                mup[:, 64:65], OP.add)
                    emit_chunks(8 if s == 0 else 0)

                    pts = p1cache.pop(g_seg)
                    # ---- phase 3: per-unit mem/rsum + dot/den + blend ----
                    for u in range(2):
                        hA, hB = 2 * u, 2 * u + 1
                        sqC = sqC2[:, s, u, :]
                        ptA, ptB = pts[u]
                        mup = mups[u]
                        memp = psp.tile([128, 512], f32, tag="att", bufs=5,
                                        name=f"mem{g_seg}{u}")
                        rsmp = psp.tile([128, 512], f32, tag="att", bufs=5,
                                        name=f"rsm{g_seg}{u}")
                        nc.tensor.matmul(memp[0:64, :], lhsT=memt[0:64, u, :],
                                         rhs=sqC[0:64, :], start=True, stop=True)
                        nc.tensor.matmul(memp[64:128, :], lhsT=memt[64:128, u, :],
                                         rhs=sqC[64:128, :], start=True, stop=True,
                                         tile_position=(64, 64))
                        nc.tensor.matmul(rsmp[0:64, :], lhsT=ones_h[0:64, :],
                                         rhs=sqC[0:64, :], start=True, stop=True)
                        nc.tensor.matmul(rsmp[64:128, :], lhsT=ones_h[64:128, :],
                                         rhs=sqC[64:128, :], start=True, stop=True,
                                         tile_position=(64, 64))
                        # mem/Z state update for next segment
                        nc.vector.tensor_tensor(memF[:, u, :], memF[:, u, :],
                                                mup[:, 0:64], OP.add)
                        nc.gpsimd.tensor_copy(memt[:, u, :], memF[:, u, :])
                        # gZ[p] = g[p] / Zt[p]; 1/rsum via fast approx
                        rzt = wp.tile([128, 1], f32, tag="rzt", bufs=2,
                                      name=f"rzt{g_seg}{u}")
                        nc.vector.reciprocal_approx_fast(rzt, Zt[:, u:u + 1])
                        gZ = wp.tile([128, 1], f32, tag="gZ", bufs=2,
                                     name=f"gZ{g_seg}{u}")
                        nc.vector.tensor_tensor(gZ, gates[:, u, 0:1], rzt,
                                                OP.mult)
                        recR = wp.tile([128, 512], f32, tag="rz", bufs=2,
                                       name=f"rR{g_seg}{u}")
                        nc.vector.reciprocal_approx_fast(recR, rsmp)
                        # dot + den matmuls
                        dotp = psp.tile([128, 512], f32, tag="att", bufs=5,
                                        name=f"dot{g_seg}{u}")
                        denp = psp.tile([128, 512], f32, tag="att", bufs=5,
                                        name=f"den{g_seg}{u}")
                        for tc_i in range(4):
                            st = (tc_i == 0)
                            sp = (tc_i == 3)
                            hs, cg = tc_i // 2, tc_i % 2
                            vA = vn2[:, hs, s, 2 * u, cg]
                            vB = vn2[:, hs, s, 2 * u + 1, cg]
                            nc.tensor.matmul(dotp[0:64, :], lhsT=vA[:, 0:64],
                                             rhs=ptA[:, tc_i], start=st, stop=sp)
                            nc.tensor.matmul(dotp[64:128, :], lhsT=vB[:, 0:64],
                                             rhs=ptB[:, tc_i], start=st, stop=sp,
                                             tile_position=(0, 64))
                            nc.tensor.matmul(denp[0:64, :], lhsT=ones_h,
                                             rhs=ptA[:, tc_i], start=st, stop=sp)
                            nc.tensor.matmul(denp[64:128, :], lhsT=ones_h,
                                             rhs=ptB[:, tc_i], start=st, stop=sp,
                                             tile_position=(0, 64))
                        # blend
                        recD = wp.tile([128, 512], f32, tag="recD", bufs=2,
                                       name=f"rD{g_seg}{u}")
                        nc.vector.reciprocal_approx_fast(recD, denp)
                        t1b = wp.tile([128, 512], f32, tag="t1b", bufs=2,
                                      name=f"t1b{g_seg}{u}")
                        nc.vector.scalar_tensor_tensor(
                            t1b, dotp, gates[:, u, 1:2], recD, OP.mult, OP.mult)
                        t2b = wp.tile([128, 512], f32, tag="t2b", bufs=2,
                                      name=f"t2b{g_seg}{u}")
                        nc.vector.scalar_tensor_tensor(
                            t2b, memp, gZ, recR, OP.mult, OP.mult)
                        attU = wp.tile([128, 512], fp16, tag="attU",
                                       name=f"aU{g_seg}{u}")
                        nc.vector.tensor_tensor(attU, t1b, t2b, OP.add)
                        # assembly into attBT
                        rcA = slice(hA * 32, hA * 32 + 32)
                        rcB = slice(hB * 32, hB * 32 + 32)
                        nc.gpsimd.tensor_copy(
                            attBT[0:64, :, rcA],
                            attU[0:64, 0:256].rearrange("p (c i) -> p c i", c=8))
                        nc.gpsimd.tensor_copy(
                            attBT[64:128, :, rcB],
                            attU[64:128, 256:512].rearrange("p (c i) -> p c i",
                                                            c=8))
                        nc.sync.dma_start(
                            attBT[64:128, :, rcA],
                            attU[0:64, 256:512].rearrange("p (c i) -> p c i",
                                                          c=8))
                        nc.sync.dma_start(
                            attBT[0:64, :, rcB],
                            attU[64:128, 0:256].rearrange("p (c i) -> p c i",
                                                          c=8))
                        emit_chunks(4 if s == 0 else 0)
                    pending_out.append((attBT, g_seg))

            while pending_out:
                do_outproj()

    nc.compile()
    return nc


def _prep_inputs(x, Wq, Wk, Wv, Wout, betas):
    gate = 1.0 / (1.0 + np.exp(-np.asarray(betas, np.float32)))[0, :, 0, :]  # (H, DV)
    wqkv = np.ascontiguousarray(
        np.stack([Wq.T, Wk.T, Wv.T]).astype(np.float16))
    wout = np.ascontiguousarray(Wout.T.astype(np.float16))
    in_maps = []
    for c in range(NCORES):
        b, h0 = c // 4, HPC * (c % 4)
        xs = x[b].reshape(NSEG, SEG, D)[:, h0 * 32:h0 * 32 + 128, :]  # (16,128,D)
        xsT = np.ascontiguousarray(xs.transpose(0, 2, 1))             # (16,D,128)
        xst = np.ascontiguousarray(
            xsT.reshape(NSB, 2, D, 128).transpose(0, 2, 1, 3).reshape(NSB, D, 256)
        ).astype(np.float16)
        gts = np.zeros((128, 2, 2), np.float32)
        for u in range(2):
            for p in range(128):
                hd = h0 + 2 * u + p // 64
                gts[p, u, 0] = gate[hd, p % 64]
                gts[p, u, 1] = 1.0 - gate[hd, p % 64]
        in_maps.append({"xst": xst, "wqkv": wqkv, "wout": wout, "gates": gts})
    return in_maps


def kernel(x, Wq, Wk, Wv, Wout, betas):
    from concourse.bass_utils import run_bass_kernel_spmd

    x = np.asarray(x, np.float32)
    in_maps = _prep_inputs(x, np.asarray(Wq), np.asarray(Wk), np.asarray(Wv),
                           np.asarray(Wout), betas)
    if "nc" not in _cache:
        _cache["nc"] = _build_program()
    res = run_bass_kernel_spmd(_cache["nc"], in_maps, core_ids=list(range(NCORES)))
    out = np.zeros((B, T, D), np.float32)
    for c in range(NCORES):
        b, h0 = c // 4, HPC * (c % 4)
        yc = res.results[c]["y"]  # (16, 128, D)
        for g in range(NSEG):
            r0 = g * SEG + h0 * 32
            out[b, r0:r0 + 128, :] = yc[g]
    return out

